# revision 6
# baseline (speedup 1.0000x reference)
"""Channel-attention (CAM) kernel for Trainium2, 8 NeuronCores.

Reference computation (per batch b):
    A   = x[b].reshape(L, C)            # L = 48^3 = 110592, C = 256
    G   = A^T A                          # [C, C] Gram matrix
    S   = softmax(G, axis=-1)
    out = gamma * (A @ S) + x[b]

Sharding: L-parallel across 8 cores; each core holds 13824 rows of each
batch (28.3 MB read + 28.3 MB write per core = the DMA floor ~160 us).

v2 design (vs. the bf16 I-fold baseline at ~236 us):
  * All PE work in fp8 (e4m3) with DoubleRow double-pumping: one matmul
    contracts 256 rows (2 k-tiles).  Gram = 12 matmuls/supertile, Y = 12.
    Transposes (A^T tiles for the Y pass) also run in fp8 with bf16-free
    drains.  Phase-1 PE (~4.3 us/supertile) now fits under the read
    stream (~4.4 us/supertile), so reads are never PE-gated.
  * Residual is NOT folded into the matmul (fp8 would wreck it):
    out = (A8 @ gamma*S8) + A_bf16, the add fused into the PSUM drain
    (vector/gpsimd tensor_add).  Softmax is saturated (Gram diag ~L
    >> off-diag ~sqrt(L)) so fp8/bf16 Gram precision is irrelevant.
  * One AllReduce per batch (128 KB bf16).  A zero-cost dummy AllReduce
    triggers at t~2us so the runtime's one-time rank-sync barrier
    (~start-skew + 15 us) burns under the read stream, not before AR0.
  * AR1's trigger is gated on AR0's completion (a globally synced event)
    so its entry barrier doesn't re-pay rank skew.
  * Reads ride the sync queue (HW q1), stores the scalar queue (HW q10):
    b0 stores overlap the tail of the read stream without queue coupling.

Engine budget per supertile (target cadence = read/store 4.4 us):
  tensor: gram 12xDR (1.6) + 24 transposes (2.7)   [phase 1]
          Y 12xDR (1.6)                            [phase 2]
  vector: f32->bf16 casts (1.9) + bf16->fp8 (1.0)  [phase 1]
          residual drains 6x687ns (4.1; b1 splits with gpsimd)
  scalar: A^T drains 2x (2-3)                      [phase 1]
          store issue + softmax                    [phase 2]
  gpsimd: collectives only until AR1 completes, then b1 drains
"""

import numpy as np
from contextlib import ExitStack

import concourse.bass as bass
import concourse.tile as tile
from concourse import bacc, mybir
from concourse.bass import ts
from concourse.bass_utils import run_bass_kernel_spmd
from concourse.masks import make_identity

F32 = mybir.dt.float32
BF16 = mybir.dt.bfloat16
FP8 = mybir.dt.float8e4
AF = mybir.ActivationFunctionType
DR = mybir.MatmulPerfMode.DoubleRow

N_CORES = 8
B = 2
L = 48 * 48 * 48          # 110592
C = 256
L_SH = L // N_CORES       # 13824 rows per core per batch
ROWS = B * L_SH           # 27648 rows per core
P = 128
RPP = 12                  # rows per partition per supertile
HPP = RPP // 2            # rows per partition per half-supertile
SROWS = P * RPP           # 1536 rows per supertile
HROWS = P * HPP           # 768 rows per half-supertile
SPB = L_SH // SROWS       # 9 supertiles per batch
S_TOT = B * SPB           # 18 supertiles per core

_CACHE: dict = {}


def _build():
    nc = bacc.Bacc(
        "TRN2", target_bir_lowering=False, debug=False, num_devices=N_CORES
    )
    x_dram = nc.dram_tensor("x", [ROWS, C], F32, kind="ExternalInput")
    g_dram = nc.dram_tensor("gamma", [1, 1], F32, kind="ExternalInput")
    o_dram = nc.dram_tensor("out", [ROWS, C], F32, kind="ExternalOutput")
    cc_in = [
        nc.dram_tensor(f"cc_in{g}", [2 * P, C], BF16, kind="Internal")
        for g in range(B)
    ]
    cc_out = [
        nc.dram_tensor(
            f"cc_out{g}", [2 * P, C], BF16, kind="Internal",
            addr_space="Shared",
        )
        for g in range(B)
    ]
    # dummy collective: triggers immediately so the runtime's one-time
    # rank-sync barrier overlaps the read stream (contents unused)
    dmy_in = nc.dram_tensor("dmy_in", [1, 16], BF16, kind="Internal")
    dmy_out = nc.dram_tensor(
        "dmy_out", [1, 16], BF16, kind="Internal", addr_space="Shared"
    )
    X, GAM, OUT = x_dram.ap(), g_dram.ap(), o_dram.ap()
    GROUPS = [list(range(N_CORES))]

    def x_half(s, h):
        # partition p holds rows s*SROWS + h*HROWS + p*HPP + (0..HPP-1)
        r0 = s * SROWS + h * HROWS
        return X[r0 : r0 + HROWS, :].rearrange("(p j) c -> p j c", j=HPP)

    def o_half(s, h):
        r0 = s * SROWS + h * HROWS
        return OUT[r0 : r0 + HROWS, :].rearrange("(p j) c -> p j c", j=HPP)

    with tile.TileContext(nc) as tc, ExitStack() as octx:
        constp = octx.enter_context(tc.tile_pool(name="const", bufs=1))
        identb = constp.tile([P, P], BF16, name="identb", tag="identb")
        make_identity(nc, identb[:])
        gam_sb = constp.tile([1, 1], F32, name="gam_sb", tag="gam_sb")
        nc.sync.dma_start(gam_sb[:], GAM[:, :])
        gam_bc = constp.tile([P, 1], F32, name="gam_bc", tag="gam_bc")
        nc.gpsimd.partition_broadcast(gam_bc[:], gam_sb[:])
        # m8[b] = gamma * softmax(G_b) as [c_part, c_blk, out_ch] fp8
        m8 = [
            constp.tile([P, 2, C], FP8, name=f"m8_{b}", tag=f"m8_{b}")
            for b in range(B)
        ]

        # dummy AR: soak runtime barrier + start skew under the reads
        nc.gpsimd.collective_compute(
            "AllReduce",
            mybir.AluOpType.add,
            replica_groups=GROUPS,
            ins=[dmy_in.ap()[:, :]],
            outs=[dmy_out.ap()[:, :]],
        )

        xbp = octx.enter_context(tc.tile_pool(name="xb", bufs=S_TOT))
        atp = octx.enter_context(tc.tile_pool(name="at", bufs=S_TOT))
        xfp = octx.enter_context(tc.tile_pool(name="xf", bufs=2))
        x8p = octx.enter_context(tc.tile_pool(name="x8", bufs=2))
        otp = octx.enter_context(tc.tile_pool(name="ot", bufs=2))
        gsp = octx.enter_context(tc.tile_pool(name="gs", bufs=2))
        smp = octx.enter_context(tc.tile_pool(name="smx", bufs=1))
        psg = octx.enter_context(tc.tile_pool(name="psg", bufs=1, space="PSUM"))
        pst = octx.enter_context(tc.tile_pool(name="pst", bufs=2, space="PSUM"))
        psy = octx.enter_context(tc.tile_pool(name="psy", bufs=3, space="PSUM"))

        g_ps = [
            psg.tile([P, 2, C], F32, name=f"g{b}", tag=f"g{b}")
            for b in range(B)
        ]
        xbs: dict = {}
        ats: dict = {}

        def load_cast(s):
            xb = xbp.tile([P, RPP, C], BF16, name="xb", tag="xb")
            xbs[s] = xb
            for h in range(2):
                xf = xfp.tile([P, HPP, C], F32, name="xf", tag="xf")
                nc.sync.dma_start(xf[:], x_half(s, h))
                nc.vector.tensor_copy(xb[:, ts(h, HPP), :], xf[:])
            x8 = x8p.tile([P, RPP, C], FP8, name="x8", tag="x8")
            nc.vector.tensor_copy(x8[:], xb[:])
            return x8

        def gram(s, x8):
            b = s // SPB
            s_in_b = s % SPB
            for jp in range(HPP):
                first = s_in_b == 0 and jp == 0
                last = s_in_b == SPB - 1 and jp == HPP - 1
                for m in range(2):
                    nc.tensor.matmul(
                        g_ps[b][:, m, :],
                        x8[:, 2 * jp : 2 * jp + 2, ts(m, P)],
                        x8[:, 2 * jp : 2 * jp + 2, :],
                        start=first, stop=last,
                        perf_mode=DR,
                    )

        def tp_work(s):
            # bf16 PE transposes (fp8 transpose mode has an output-step-2
            # HW restriction); the scalar drain converts to fp8
            xb = xbs[s]
            at = atp.tile([P, RPP, 2, P], FP8, name="at", tag="at")
            ats[s] = at
            for q4 in range(4):
                tpp = pst.tile([P, 3, 2, P], BF16, name="tpp", tag="tpp")
                for jj in range(3):
                    j = 3 * q4 + jj
                    for blk in range(2):
                        nc.tensor.transpose(
                            tpp[:, jj, blk, :],
                            xb[:, j, ts(blk, P)],
                            identb[:],
                        )
                nc.scalar.copy(at[:, 3 * q4 : 3 * q4 + 3, :, :], tpp[:])

        def stage_and_ar(g):
            gsb = gsp.tile([P, 2, C], BF16, name="gsb", tag="gsb")
            nc.vector.tensor_copy(gsb[:], g_ps[g][:])
            for m in range(2):
                nc.gpsimd.dma_start(cc_in[g].ap()[ts(m, P), :], gsb[:, m, :])
            nc.gpsimd.collective_compute(
                "AllReduce",
                mybir.AluOpType.add,
                replica_groups=GROUPS,
                ins=[cc_in[g].ap()[:, :]],
                outs=[cc_out[g].ap()[:, :]],
            )

        def softmax(b):
            gf = smp.tile([P, 2, C], BF16, name=f"gf{b}", tag=f"gf{b}")
            for m in range(2):
                nc.scalar.dma_start(gf[:, m, :], cc_out[b].ap()[ts(m, P), :])
            for m in range(2):
                nmx = smp.tile([P, 1], F32, name="nmx", tag="nmx")
                nc.vector.tensor_reduce(
                    nmx[:], gf[:, m, :],
                    axis=mybir.AxisListType.X,
                    op=mybir.AluOpType.max,
                    negate=True,
                )
                ex = smp.tile([P, C], F32, name="ex", tag="ex")
                ssum = smp.tile([P, 1], F32, name="ssum", tag="ssum")
                nc.scalar.activation(
                    ex[:], gf[:, m, :], AF.Exp, bias=nmx[:], scale=1.0,
                    accum_out=ssum[:],
                )
                inv = smp.tile([P, 1], F32, name="inv", tag="inv")
                nc.vector.reciprocal(inv[:], ssum[:])
                sc = smp.tile([P, 1], F32, name="sc", tag="sc")
                nc.vector.tensor_mul(sc[:], inv[:], gam_bc[:])
                nc.scalar.activation(m8[b][:, m, :], ex[:], AF.Copy, scale=sc[:])

        def ywork(s):
            b, at, xb = s // SPB, ats[s], xbs[s]
            for h in range(2):
                ot = otp.tile([P, HPP, C], F32, name="ot", tag="ot")
                for jj3 in range(HPP // 2):
                    y = psy.tile([P, 2, C], F32, name="y", tag="y")
                    for q in range(2):
                        j = h * HPP + 2 * jj3 + q
                        nc.tensor.matmul(
                            y[:, q, :], at[:, j, :, :], m8[b][:],
                            start=True, stop=True,
                            perf_mode=DR,
                        )
                    # residual fused into the PSUM drain (gpsimd cannot
                    # read PSUM, so vector carries all of these)
                    jlo = h * HPP + 2 * jj3
                    nc.vector.tensor_add(
                        ot[:, ts(jj3, 2), :], y[:], xb[:, jlo : jlo + 2, :]
                    )
                nc.scalar.dma_start(o_half(s, h), ot[:])

        # ---------------- phase 1 ----------------
        for s in range(S_TOT):
            x8 = load_cast(s)
            gram(s, x8)
            tp_work(s)
            if s == SPB - 1:
                stage_and_ar(0)
            if s == S_TOT - 1:
                # gate AR1's trigger on AR0 COMPLETION (globally synced)
                # so its entry barrier doesn't re-pay the start skew
                ccw = gsp.tile([1, C], BF16, name="ccw", tag="ccw")
                nc.gpsimd.dma_start(ccw[:], cc_out[0].ap()[0:1, :])
                stage_and_ar(1)

        # ---------------- phase 2 ----------------
        softmax(0)
        for s in range(SPB):
            ywork(s)
        softmax(1)
        for s in range(SPB, S_TOT):
            ywork(s)

    nc.compile()
    return nc


def _get_nc():
    if "nc" not in _CACHE:
        _CACHE["nc"] = _build()
    return _CACHE["nc"]


def kernel(x: np.ndarray, gamma: np.ndarray, **_kw) -> np.ndarray:
    nc = _get_nc()
    x = np.asarray(x, dtype=np.float32)
    orig_shape = x.shape
    x3 = x.reshape(B, L, C)
    gam = np.asarray(gamma, dtype=np.float32).reshape(1, 1)
    in_maps = []
    for k in range(N_CORES):
        shard = np.ascontiguousarray(
            x3[:, k * L_SH : (k + 1) * L_SH, :]
        ).reshape(ROWS, C)
        in_maps.append({"x": shard, "gamma": gam})
    res = run_bass_kernel_spmd(nc, in_maps, core_ids=list(range(N_CORES)))
    out = np.empty((B, L, C), dtype=np.float32)
    for k in range(N_CORES):
        out[:, k * L_SH : (k + 1) * L_SH, :] = res.results[k]["out"].reshape(
            B, L_SH, C
        )
    return out.reshape(orig_shape)


# revision 8
# speedup vs baseline: 1.0590x; 1.0590x over previous
"""Channel-attention (CAM) kernel for Trainium2, 8 NeuronCores.

Reference computation (per batch b):
    A   = x[b].reshape(L, C)            # L = 48^3 = 110592, C = 256
    G   = A^T A                          # [C, C] Gram matrix
    S   = softmax(G, axis=-1)
    out = gamma * (A @ S) + x[b]

Algebraic fold: out = A @ (gamma*S + I) since A @ I == x.  This removes
the residual add AND the second read of x: HBM traffic is the floor
(read 28.3 MB + write 28.3 MB per core).  A^T (bf16) stays resident in
SBUF between the phases.

v3 schedule (measured findings from the v0/v2 traces):
  * A^T is built with `is_transpose` PE transposes (~65 ns per 128x128
    tile, half the cost of the identity-matmul transposes) writing
    bf16 PSUM, drained in 3-tile batches by the scalar engine.  fp8 /
    DoubleRow was tried and abandoned: on HW a DoubleRow matmul streams
    columns at the same rate as bf16, so fp8 buys no time here.
  * One AllReduce per Gram (128 KB bf16): AR0 after batch-0's reads
    (~55 us), AR1 after batch-1's.  AR1's trigger is gated on AR0's
    completion (a globally synced event) so its entry barrier doesn't
    re-pay rank start-skew.  The runtime's one-time rank barrier
    (~25-60 us) overlaps the read stream and only floors AR0's start.
  * Reads alternate the sync/vector DMA queues and stores alternate
    the sync/scalar queues (one HW queue tops out ~260-320 GB/s; two
    get closer to the 358 GB/s core limit).  xf/ot pools are 4 deep so
    buffer recycling never gates the streams.
  * Engine budget per supertile (cadence target = DMA ~4.6 us):
    tensor: gram 24x131ns + 24 transposes x65ns = 4.7 us   [phase 1]
            Y 24x131ns = 3.1 us                            [phase 2]
    vector: two f32->bf16 casts = 1.9 us + 1 read issue    [phase 1]
            3 of 6 Y-PSUM drains = 2.1 us                  [phase 2]
    scalar: 4 A^T drains = 3.6 us                          [phase 1]
            3 Y drains + 1 store issue                     [phase 2]
    gpsimd: collective staging + triggers only
"""

import numpy as np
from contextlib import ExitStack

import concourse.bass as bass
import concourse.tile as tile
from concourse import bacc, mybir
from concourse.bass import ts
from concourse.bass_utils import run_bass_kernel_spmd
from concourse.masks import make_identity

F32 = mybir.dt.float32
BF16 = mybir.dt.bfloat16
AF = mybir.ActivationFunctionType

N_CORES = 8
B = 2
L = 48 * 48 * 48          # 110592
C = 256
L_SH = L // N_CORES       # 13824 rows per core per batch
ROWS = B * L_SH           # 27648 rows per core
P = 128
RPP = 12                  # rows per partition per supertile
HPP = RPP // 2            # rows per partition per half-supertile
SROWS = P * RPP           # 1536 rows per supertile
HROWS = P * HPP           # 768 rows per half-supertile
SPB = L_SH // SROWS       # 9 supertiles per batch
S_TOT = B * SPB           # 18 supertiles per core

_CACHE: dict = {}


def _build():
    nc = bacc.Bacc(
        "TRN2", target_bir_lowering=False, debug=False, num_devices=N_CORES
    )
    x_dram = nc.dram_tensor("x", [ROWS, C], F32, kind="ExternalInput")
    g_dram = nc.dram_tensor("gamma", [1, 1], F32, kind="ExternalInput")
    o_dram = nc.dram_tensor("out", [ROWS, C], F32, kind="ExternalOutput")
    cc_in = [
        nc.dram_tensor(f"cc_in{g}", [2 * P, C], BF16, kind="Internal")
        for g in range(B)
    ]
    cc_out = [
        nc.dram_tensor(
            f"cc_out{g}", [2 * P, C], BF16, kind="Internal",
            addr_space="Shared",
        )
        for g in range(B)
    ]
    X, GAM, OUT = x_dram.ap(), g_dram.ap(), o_dram.ap()
    GROUPS = [list(range(N_CORES))]

    def x_half(s, h):
        # partition p holds rows s*SROWS + h*HROWS + p*HPP + (0..HPP-1)
        r0 = s * SROWS + h * HROWS
        return X[r0 : r0 + HROWS, :].rearrange("(p j) c -> p j c", j=HPP)

    def o_half(s, h):
        r0 = s * SROWS + h * HROWS
        return OUT[r0 : r0 + HROWS, :].rearrange("(p j) c -> p j c", j=HPP)

    with tile.TileContext(nc) as tc, ExitStack() as octx:
        constp = octx.enter_context(tc.tile_pool(name="const", bufs=1))
        identb = constp.tile([P, P], BF16, name="identb", tag="identb")
        make_identity(nc, identb[:])
        gam_sb = constp.tile([1, 1], F32, name="gam_sb", tag="gam_sb")
        nc.sync.dma_start(gam_sb[:], GAM[:, :])
        gam_bc = constp.tile([P, 1], F32, name="gam_bc", tag="gam_bc")
        nc.gpsimd.partition_broadcast(gam_bc[:], gam_sb[:])
        # m_bf[2b+q] = gamma * softmax(G_b)[qP:(q+1)P, :] + I-block
        m_bf = [
            constp.tile([P, C], BF16, name=f"mbf{i}", tag=f"mbf{i}")
            for i in range(4)
        ]

        atp = octx.enter_context(tc.tile_pool(name="at", bufs=S_TOT))
        xbp = octx.enter_context(tc.tile_pool(name="xb", bufs=3))
        xfp = octx.enter_context(tc.tile_pool(name="xf", bufs=4))
        otp = octx.enter_context(tc.tile_pool(name="ot", bufs=4))
        gsp = octx.enter_context(tc.tile_pool(name="gs", bufs=2))
        smp = octx.enter_context(tc.tile_pool(name="smx", bufs=1))
        psg = octx.enter_context(tc.tile_pool(name="psg", bufs=1, space="PSUM"))
        pst = octx.enter_context(tc.tile_pool(name="pst", bufs=3, space="PSUM"))
        psy = octx.enter_context(tc.tile_pool(name="psy", bufs=3, space="PSUM"))

        g_ps = [
            psg.tile([P, 2, C], F32, name=f"g{b}", tag=f"g{b}")
            for b in range(B)
        ]
        ats: dict = {}

        def load_cast(s):
            xb = xbp.tile([P, RPP, C], BF16, name="xb", tag="xb")
            for h in range(2):
                xf = xfp.tile([P, HPP, C], F32, name="xf", tag="xf")
                # reads alternate two HW DMA queues (DVE can't issue DMAs)
                eng = nc.sync if h == 0 else nc.scalar
                eng.dma_start(xf[:], x_half(s, h))
                nc.vector.tensor_copy(xb[:, ts(h, HPP), :], xf[:])
            return xb

        def gram(s, xb):
            b = s // SPB
            s_in_b = s % SPB
            for j in range(RPP):
                first = s_in_b == 0 and j == 0
                last = s_in_b == SPB - 1 and j == RPP - 1
                for m in range(2):
                    nc.tensor.matmul(
                        g_ps[b][:, m, :], xb[:, j, ts(m, P)], xb[:, j, :],
                        start=first, stop=last,
                    )

        def tp_work(s, xb):
            at = atp.tile([P, RPP, 2, P], BF16, name="at", tag="at")
            ats[s] = at
            for q4 in range(4):
                tpp = pst.tile([P, 3, 2, P], BF16, name="tpp", tag="tpp")
                for jj in range(3):
                    j = 3 * q4 + jj
                    for blk in range(2):
                        nc.tensor.transpose(
                            tpp[:, jj, blk, :],
                            xb[:, j, ts(blk, P)],
                            identb[:],
                        )
                nc.scalar.copy(at[:, 3 * q4 : 3 * q4 + 3, :, :], tpp[:])

        def stage_and_ar(g):
            gsb = gsp.tile([P, 2, C], BF16, name="gsb", tag="gsb")
            nc.vector.tensor_copy(gsb[:], g_ps[g][:])
            for m in range(2):
                nc.gpsimd.dma_start(cc_in[g].ap()[ts(m, P), :], gsb[:, m, :])
            nc.gpsimd.collective_compute(
                "AllReduce",
                mybir.AluOpType.add,
                replica_groups=GROUPS,
                ins=[cc_in[g].ap()[:, :]],
                outs=[cc_out[g].ap()[:, :]],
            )

        def softmax(b):
            gf = smp.tile([P, 2, C], BF16, name=f"gf{b}", tag=f"gf{b}")
            for m in range(2):
                nc.scalar.dma_start(gf[:, m, :], cc_out[b].ap()[ts(m, P), :])
            for m in range(2):
                i = 2 * b + m
                nmx = smp.tile([P, 1], F32, name="nmx", tag="nmx")
                nc.vector.tensor_reduce(
                    nmx[:], gf[:, m, :],
                    axis=mybir.AxisListType.X,
                    op=mybir.AluOpType.max,
                    negate=True,
                )
                ex = smp.tile([P, C], F32, name="ex", tag="ex")
                ssum = smp.tile([P, 1], F32, name="ssum", tag="ssum")
                nc.scalar.activation(
                    ex[:], gf[:, m, :], AF.Exp, bias=nmx[:], scale=1.0,
                    accum_out=ssum[:],
                )
                inv = smp.tile([P, 1], F32, name="inv", tag="inv")
                nc.vector.reciprocal(inv[:], ssum[:])
                sc = smp.tile([P, 1], F32, name="sc", tag="sc")
                nc.vector.tensor_mul(sc[:], inv[:], gam_bc[:])
                nc.scalar.activation(m_bf[i][:], ex[:], AF.Copy, scale=sc[:])
                # fold the residual: M = gamma*S + I (diagonal block m)
                nc.vector.tensor_add(
                    m_bf[i][:, ts(m, P)], m_bf[i][:, ts(m, P)], identb[:]
                )

        def ywork(s):
            b, at = s // SPB, ats[s]
            for h in range(2):
                ot = otp.tile([P, HPP, C], F32, name="ot", tag="ot")
                for jj3 in range(HPP // 2):
                    y = psy.tile([P, 2, C], F32, name="y", tag="y")
                    for q in range(2):
                        j = h * HPP + 2 * jj3 + q
                        nc.tensor.matmul(
                            y[:, q, :], at[:, j, 0, :], m_bf[2 * b][:],
                            start=True, stop=False,
                        )
                        nc.tensor.matmul(
                            y[:, q, :], at[:, j, 1, :], m_bf[2 * b + 1][:],
                            start=False, stop=True,
                        )
                    dst = ot[:, ts(jj3, 2), :]
                    if jj3 == 1:
                        nc.scalar.activation(dst, y[:], AF.Copy)
                    else:
                        nc.vector.tensor_copy(dst, y[:])
                # stores alternate two HW DMA queues (sync is free now)
                eng = nc.scalar if h == 0 else nc.sync
                eng.dma_start(o_half(s, h), ot[:])

        # ---------------- phase 1 ----------------
        for s in range(S_TOT):
            xb = load_cast(s)
            gram(s, xb)
            tp_work(s, xb)
            if s == SPB - 1:
                stage_and_ar(0)
            if s == S_TOT - 1:
                # gate AR1's trigger on AR0 COMPLETION (globally synced)
                # so its entry barrier doesn't re-pay the start skew
                ccw = gsp.tile([1, C], BF16, name="ccw", tag="ccw")
                nc.gpsimd.dma_start(ccw[:], cc_out[0].ap()[0:1, :])
                stage_and_ar(1)

        # ---------------- phase 2 ----------------
        softmax(0)
        for s in range(SPB):
            ywork(s)
        softmax(1)
        for s in range(SPB, S_TOT):
            ywork(s)

    nc.compile()
    return nc


def _get_nc():
    if "nc" not in _CACHE:
        _CACHE["nc"] = _build()
    return _CACHE["nc"]


def kernel(x: np.ndarray, gamma: np.ndarray, **_kw) -> np.ndarray:
    nc = _get_nc()
    x = np.asarray(x, dtype=np.float32)
    orig_shape = x.shape
    x3 = x.reshape(B, L, C)
    gam = np.asarray(gamma, dtype=np.float32).reshape(1, 1)
    in_maps = []
    for k in range(N_CORES):
        shard = np.ascontiguousarray(
            x3[:, k * L_SH : (k + 1) * L_SH, :]
        ).reshape(ROWS, C)
        in_maps.append({"x": shard, "gamma": gam})
    res = run_bass_kernel_spmd(nc, in_maps, core_ids=list(range(N_CORES)))
    out = np.empty((B, L, C), dtype=np.float32)
    for k in range(N_CORES):
        out[:, k * L_SH : (k + 1) * L_SH, :] = res.results[k]["out"].reshape(
            B, L_SH, C
        )
    return out.reshape(orig_shape)


# revision 14
# speedup vs baseline: 1.1237x; 1.0611x over previous
"""Channel-attention (CAM) kernel for Trainium2, 8 NeuronCores.

Reference computation (per batch b):
    A   = x[b].reshape(L, C)            # L = 48^3 = 110592, C = 256
    G   = A^T A                          # [C, C] Gram matrix
    S   = softmax(G, axis=-1)
    out = gamma * (A @ S) + x[b]

Algebraic fold: out = A @ (gamma*S + I) since A @ I == x.  This removes
the residual add AND the second read of x: HBM traffic is the floor
(read 28.3 MB + write 28.3 MB per core).  A^T (bf16) stays resident in
SBUF between the phases.

v3 schedule (measured findings from the v0/v2 traces):
  * A^T is built with `is_transpose` PE transposes (~65 ns per 128x128
    tile, half the cost of the identity-matmul transposes) writing
    bf16 PSUM, drained in 3-tile batches by the scalar engine.  fp8 /
    DoubleRow was tried and abandoned: on HW a DoubleRow matmul streams
    columns at the same rate as bf16, so fp8 buys no time here.
  * One AllReduce per Gram (128 KB bf16): AR0 after batch-0's reads
    (~55 us), AR1 after batch-1's.  AR1's trigger is gated on AR0's
    completion (a globally synced event) so its entry barrier doesn't
    re-pay rank start-skew.  The runtime's one-time rank barrier
    (~25-60 us) overlaps the read stream and only floors AR0's start.
  * Reads alternate the sync/vector DMA queues and stores alternate
    the sync/scalar queues (one HW queue tops out ~260-320 GB/s; two
    get closer to the 358 GB/s core limit).  xf/ot pools are 4 deep so
    buffer recycling never gates the streams.
  * Engine budget per supertile (cadence target = DMA ~4.6 us):
    tensor: gram 24x131ns + 24 transposes x65ns = 4.7 us   [phase 1]
            Y 24x131ns = 3.1 us                            [phase 2]
    vector: two f32->bf16 casts = 1.9 us + 1 read issue    [phase 1]
            3 of 6 Y-PSUM drains = 2.1 us                  [phase 2]
    scalar: 4 A^T drains = 3.6 us                          [phase 1]
            3 Y drains + 1 store issue                     [phase 2]
    gpsimd: collective staging + triggers only
"""

import numpy as np
from contextlib import ExitStack

import concourse.bass as bass
import concourse.tile as tile
from concourse import bacc, mybir
from concourse.bass import ts
from concourse.bass_utils import run_bass_kernel_spmd
from concourse.masks import make_identity

F32 = mybir.dt.float32
BF16 = mybir.dt.bfloat16
FP16 = mybir.dt.float16
AF = mybir.ActivationFunctionType
# Gram entries reach ~L (110592) > fp16 max; staged as G/4 (max ~27.6k)
# and rescaled exactly inside the softmax (exp(4*g4 - 4*max4)).
AR_SCALE = 0.25

N_CORES = 8
B = 2
L = 48 * 48 * 48          # 110592
C = 256
L_SH = L // N_CORES       # 13824 rows per core per batch
ROWS = B * L_SH           # 27648 rows per core
P = 128
RPP = 12                  # rows per partition per supertile
HPP = RPP // 2            # rows per partition per half-supertile
SROWS = P * RPP           # 1536 rows per supertile
HROWS = P * HPP           # 768 rows per half-supertile
SPB = L_SH // SROWS       # 9 supertiles per batch
S_TOT = B * SPB           # 18 supertiles per core

_CACHE: dict = {}


def _build():
    nc = bacc.Bacc(
        "TRN2", target_bir_lowering=False, debug=False, num_devices=N_CORES
    )
    x_dram = nc.dram_tensor("x", [ROWS, C], F32, kind="ExternalInput")
    g_dram = nc.dram_tensor("gamma", [1, 1], F32, kind="ExternalInput")
    o_dram = nc.dram_tensor("out", [ROWS, C], F32, kind="ExternalOutput")
    cc_in = [
        nc.dram_tensor(f"cc_in{g}", [2 * P, C], FP16, kind="Internal")
        for g in range(B)
    ]
    cc_out = [
        nc.dram_tensor(
            f"cc_out{g}", [2 * P, C], FP16, kind="Internal",
            addr_space="Shared",
        )
        for g in range(B)
    ]
    X, GAM, OUT = x_dram.ap(), g_dram.ap(), o_dram.ap()
    GROUPS = [list(range(N_CORES))]

    def x_half(s, h):
        # partition p holds rows s*SROWS + h*HROWS + p*HPP + (0..HPP-1)
        r0 = s * SROWS + h * HROWS
        return X[r0 : r0 + HROWS, :].rearrange("(p j) c -> p j c", j=HPP)

    def o_half(s, h):
        r0 = s * SROWS + h * HROWS
        return OUT[r0 : r0 + HROWS, :].rearrange("(p j) c -> p j c", j=HPP)

    with tile.TileContext(nc) as tc, ExitStack() as octx:
        constp = octx.enter_context(tc.tile_pool(name="const", bufs=1))
        identb = constp.tile([P, P], BF16, name="identb", tag="identb")
        make_identity(nc, identb[:])
        gam_sb = constp.tile([1, 1], F32, name="gam_sb", tag="gam_sb")
        nc.sync.dma_start(gam_sb[:], GAM[:, :])
        gam_bc = constp.tile([P, 1], F32, name="gam_bc", tag="gam_bc")
        nc.gpsimd.partition_broadcast(gam_bc[:], gam_sb[:])
        # m_bf[2b+q] = gamma * softmax(G_b)[qP:(q+1)P, :] + I-block
        m_bf = [
            constp.tile([P, C], BF16, name=f"mbf{i}", tag=f"mbf{i}")
            for i in range(4)
        ]

        atp = octx.enter_context(tc.tile_pool(name="at", bufs=S_TOT))
        xbp = octx.enter_context(tc.tile_pool(name="xb", bufs=3))
        xfp = octx.enter_context(tc.tile_pool(name="xf", bufs=6))
        otp = octx.enter_context(tc.tile_pool(name="ot", bufs=4))
        gsp = octx.enter_context(tc.tile_pool(name="gs", bufs=2))
        smp = octx.enter_context(tc.tile_pool(name="smx", bufs=1))
        psg = octx.enter_context(tc.tile_pool(name="psg", bufs=1, space="PSUM"))
        pst = octx.enter_context(tc.tile_pool(name="pst", bufs=3, space="PSUM"))
        psy = octx.enter_context(tc.tile_pool(name="psy", bufs=3, space="PSUM"))

        g_ps = [
            psg.tile([P, 2, C], F32, name=f"g{b}", tag=f"g{b}")
            for b in range(B)
        ]
        ats: dict = {}

        def load_cast(s):
            xb = xbp.tile([P, RPP, C], BF16, name="xb", tag="xb")
            for h in range(2):
                xf = xfp.tile([P, HPP, C], F32, name="xf", tag="xf")
                # all reads on the (otherwise empty) sync queue: deep
                # outstanding-DMA pipelining is what sustains ~320 GB/s
                nc.sync.dma_start(xf[:], x_half(s, h))
                nc.vector.tensor_copy(xb[:, ts(h, HPP), :], xf[:])
            return xb

        def gram(s, xb):
            b = s // SPB
            s_in_b = s % SPB
            for j in range(RPP):
                first = s_in_b == 0 and j == 0
                last = s_in_b == SPB - 1 and j == RPP - 1
                for m in range(2):
                    nc.tensor.matmul(
                        g_ps[b][:, m, :], xb[:, j, ts(m, P)], xb[:, j, :],
                        start=first, stop=last,
                    )

        def tp_work(s, xb):
            at = atp.tile([P, RPP, 2, P], BF16, name="at", tag="at")
            ats[s] = at
            for q4 in range(4):
                tpp = pst.tile([P, 3, 2, P], BF16, name="tpp", tag="tpp")
                for jj in range(3):
                    j = 3 * q4 + jj
                    for blk in range(2):
                        nc.tensor.transpose(
                            tpp[:, jj, blk, :],
                            xb[:, j, ts(blk, P)],
                            identb[:],
                        )
                nc.scalar.copy(at[:, 3 * q4 : 3 * q4 + 3, :, :], tpp[:])

        def stage_and_ar(g):
            gsb = gsp.tile([P, 2, C], FP16, name="gsb", tag="gsb")
            nc.vector.tensor_scalar_mul(gsb[:], g_ps[g][:], AR_SCALE)
            for m in range(2):
                nc.gpsimd.dma_start(cc_in[g].ap()[ts(m, P), :], gsb[:, m, :])
            nc.gpsimd.collective_compute(
                "AllReduce",
                mybir.AluOpType.add,
                replica_groups=GROUPS,
                ins=[cc_in[g].ap()[:, :]],
                outs=[cc_out[g].ap()[:, :]],
            )

        def softmax(b):
            gf = smp.tile([P, 2, C], FP16, name=f"gf{b}", tag=f"gf{b}")
            for m in range(2):
                nc.scalar.dma_start(gf[:, m, :], cc_out[b].ap()[ts(m, P), :])
            for m in range(2):
                i = 2 * b + m
                nmx = smp.tile([P, 1], F32, name="nmx", tag="nmx")
                nc.vector.tensor_reduce(
                    nmx[:], gf[:, m, :],
                    axis=mybir.AxisListType.X,
                    op=mybir.AluOpType.max,
                    negate=True,
                )
                # undo AR_SCALE exactly: exp((g - max)) = exp(4*g4 + 4*nmx)
                nmx4 = smp.tile([P, 1], F32, name="nmx4", tag="nmx4")
                nc.vector.tensor_scalar_mul(nmx4[:], nmx[:], 1.0 / AR_SCALE)
                ex = smp.tile([P, C], F32, name="ex", tag="ex")
                ssum = smp.tile([P, 1], F32, name="ssum", tag="ssum")
                nc.scalar.activation(
                    ex[:], gf[:, m, :], AF.Exp, bias=nmx4[:],
                    scale=1.0 / AR_SCALE,
                    accum_out=ssum[:],
                )
                inv = smp.tile([P, 1], F32, name="inv", tag="inv")
                nc.vector.reciprocal(inv[:], ssum[:])
                sc = smp.tile([P, 1], F32, name="sc", tag="sc")
                nc.vector.tensor_mul(sc[:], inv[:], gam_bc[:])
                nc.scalar.activation(m_bf[i][:], ex[:], AF.Copy, scale=sc[:])
                # fold the residual: M = gamma*S + I (diagonal block m)
                nc.vector.tensor_add(
                    m_bf[i][:, ts(m, P)], m_bf[i][:, ts(m, P)], identb[:]
                )

        def ywork(s):
            b, at = s // SPB, ats[s]
            for h in range(2):
                ot = otp.tile([P, HPP, C], F32, name="ot", tag="ot")
                for jj3 in range(HPP // 2):
                    y = psy.tile([P, 2, C], F32, name="y", tag="y")
                    for q in range(2):
                        j = h * HPP + 2 * jj3 + q
                        nc.tensor.matmul(
                            y[:, q, :], at[:, j, 0, :], m_bf[2 * b][:],
                            start=True, stop=False,
                        )
                        nc.tensor.matmul(
                            y[:, q, :], at[:, j, 1, :], m_bf[2 * b + 1][:],
                            start=False, stop=True,
                        )
                    dst = ot[:, ts(jj3, 2), :]
                    if jj3 == 1:
                        nc.scalar.activation(dst, y[:], AF.Copy)
                    else:
                        nc.vector.tensor_copy(dst, y[:])
                # stores alternate two HW DMA queues (sync is free now)
                eng = nc.scalar if h == 0 else nc.sync
                eng.dma_start(o_half(s, h), ot[:])

        # ---------------- phase 1 ----------------
        for s in range(S_TOT):
            xb = load_cast(s)
            gram(s, xb)
            tp_work(s, xb)
            if s == SPB - 1:
                stage_and_ar(0)
            if s == S_TOT - 1:
                # gate AR1's trigger on AR0 COMPLETION (globally synced)
                # so its entry barrier doesn't re-pay the start skew
                ccw = gsp.tile([1, C], BF16, name="ccw", tag="ccw")
                nc.gpsimd.dma_start(ccw[:], cc_out[0].ap()[0:1, :])
                stage_and_ar(1)

        # ---------------- phase 2 ----------------
        softmax(0)
        for s in range(SPB):
            ywork(s)
        softmax(1)
        for s in range(SPB, S_TOT):
            ywork(s)

    nc.compile()
    return nc


def _get_nc():
    if "nc" not in _CACHE:
        _CACHE["nc"] = _build()
    return _CACHE["nc"]


def kernel(x: np.ndarray, gamma: np.ndarray, **_kw) -> np.ndarray:
    nc = _get_nc()
    x = np.asarray(x, dtype=np.float32)
    orig_shape = x.shape
    x3 = x.reshape(B, L, C)
    gam = np.asarray(gamma, dtype=np.float32).reshape(1, 1)
    in_maps = []
    for k in range(N_CORES):
        shard = np.ascontiguousarray(
            x3[:, k * L_SH : (k + 1) * L_SH, :]
        ).reshape(ROWS, C)
        in_maps.append({"x": shard, "gamma": gam})
    res = run_bass_kernel_spmd(nc, in_maps, core_ids=list(range(N_CORES)))
    out = np.empty((B, L, C), dtype=np.float32)
    for k in range(N_CORES):
        out[:, k * L_SH : (k + 1) * L_SH, :] = res.results[k]["out"].reshape(
            B, L_SH, C
        )
    return out.reshape(orig_shape)
